# revision 54
# baseline (speedup 1.0000x reference)
"""Causal multi-head self-attention (B=2, T=2048, D=1024, H=16) on 8 TRN2
NeuronCores.

Sharding (Megatron-style, hardcoded): core = 4*b + g where b in {0,1} is the
batch and g in {0..3} a group of 4 heads. Each core computes Q/K/V projections
for its head group from x[b], fused causal attention for those 4 heads, and a
partial output projection against its 256-column slice of Wo. The host sums
the 4 partial outputs per batch (the all-reduce after out_proj).

v2 (vs the fp32r baseline):
 - All matmul operands in bf16 (1 cycle/row at any free size; accumulation
   stays fp32 in PSUM). Verified numerics: max rel err ~4e-3 vs the 2e-2 gate.
 - Inputs DMA'd as bf16 (half the bytes), finely sliced and spread over three
   SWDGE rings (sync/gpsimd/scalar) so phase 1 of chunk 0 starts ~3us in.
 - Diagonal score matmuls shrunk to the causal extent (free 512/384/256/128);
   the masked-out prefix of the exp'd tile is zeroed by a cheap Pool memset,
   and only the true 128-wide diagonal block runs affine_select.
 - Software-pipelined phase 2: PV(kti) is emitted after scores(kti+1), so the
   exp on the ACT engine is hidden behind PE work; projection (next chunk) and
   out-projection (previous chunks) matmuls are pumped as fillers between kti
   groups to keep the PE busy while ACT catches up.
 - PSUM: 2x[128,2,512] score tiles + 2x[128,512] PV accumulators +
   2x[128,512] projection tiles = exactly 8 banks.
"""

import numpy as np
import ml_dtypes

import concourse.bass as bass
import concourse.tile as tile
from concourse import bacc, mybir
from concourse.bass_utils import run_bass_kernel_spmd

B, T, D, H, DH = 2, 2048, 1024, 16, 64
HPC = 4  # heads per core
GC = 256  # projection columns per core (HPC * DH)
N_CORES = 8
F32 = mybir.dt.float32
BF16 = mybir.dt.bfloat16
EXP = mybir.ActivationFunctionType.Exp
NPBF = ml_dtypes.bfloat16

_CACHE = {}


def _build():
    nc = bacc.Bacc(
        "TRN2", target_bir_lowering=False, debug=False, num_devices=N_CORES
    )
    # Pre-swizzled inputs (host does the transposes + bf16 cast):
    #   xs[p, tc, dt, t] = x[b, tc*512+t, dt*128+p]
    #   wq/wk/wv[p, dt, c] = W[g*256+c, dt*128+p]
    #   wo[p, ct, n] = Wo[n, g*256 + ct*128 + p]
    xs = nc.dram_tensor("xs", [128, 4, 8, 512], BF16, kind="ExternalInput").ap()
    wqs = nc.dram_tensor("wqs", [128, 8, GC], BF16, kind="ExternalInput").ap()
    wks = nc.dram_tensor("wks", [128, 8, GC], BF16, kind="ExternalInput").ap()
    wvs = nc.dram_tensor("wvs", [128, 8, GC], BF16, kind="ExternalInput").ap()
    wos = nc.dram_tensor("wos", [128, 2, D], BF16, kind="ExternalInput").ap()
    # causal 0/1 mask for the diagonal 128-block, duplicated per head-of-pair:
    # tris[p, a, j] = 1.0 if j >= p else 0.0
    tris = nc.dram_tensor("tris", [128, 2, 128], BF16, kind="ExternalInput").ap()
    # partial output in bf16 (the host upcasts and sums the 4 partials; the
    # extra 2^-8 rounding on ~0.06-magnitude partials is ~3e-4 relative)
    out = nc.dram_tensor("out", [T, D], BF16, kind="ExternalOutput").ap()

    with tile.TileContext(nc) as tc:
        with (
            tc.tile_pool(name="persist", bufs=1) as persist,
            tc.tile_pool(name="xtp", bufs=4) as xtp,
            tc.tile_pool(name="ptp", bufs=4) as ptp,
            tc.tile_pool(name="normp", bufs=4) as normp,
            tc.tile_pool(name="outp", bufs=3) as outp,
            tc.tile_pool(name="pst", bufs=2, space="PSUM") as pst,
            tc.tile_pool(name="pacc", bufs=2, space="PSUM") as pacc,
            tc.tile_pool(name="pmisc", bufs=2, space="PSUM") as pmisc,
        ):
            wq = persist.tile([128, 8, GC], BF16, tag="wq")
            wk = persist.tile([128, 8, GC], BF16, tag="wk")
            wv = persist.tile([128, 8, GC], BF16, tag="wv")
            wo = persist.tile([128, 2, D], BF16, tag="wo")
            qt = persist.tile([128, 2, T], BF16, tag="qt")
            kt = persist.tile([128, 2, T], BF16, tag="kt")
            vp = persist.tile([128, 16, HPC, DH + 1], BF16, tag="vp")
            at = persist.tile([128, 2, T], BF16, tag="at")
            tri = persist.tile([128, 2, 128], BF16, tag="tri")

            # ones column of V' (row-sum trick)
            nc.gpsimd.memset(vp[:, :, :, DH], 1.0)

            # ---- prologue DMAs: fine slices over three SWDGE rings ----
            xtiles = [
                xtp.tile([128, 8, 512], BF16, tag="xt", name=f"xt{i}")
                for i in range(4)
            ]
            L = []
            for dj in range(4):
                s = slice(2 * dj, 2 * dj + 2)
                L.append((wq[:, s], wqs[:, s]))
                L.append((xtiles[0][:, 2 * dj], xs[:, 0, 2 * dj]))
                L.append((xtiles[0][:, 2 * dj + 1], xs[:, 0, 2 * dj + 1]))
            for dj in range(4):
                s = slice(2 * dj, 2 * dj + 2)
                L.append((wk[:, s], wks[:, s]))
            for dj in range(4):
                s = slice(2 * dj, 2 * dj + 2)
                L.append((wv[:, s], wvs[:, s]))
            for dj in range(4):
                s = slice(2 * dj, 2 * dj + 2)
                L.append((xtiles[1][:, s], xs[:, 1, s]))
            L.append((wo[:, 0], wos[:, 0]))
            L.append((wo[:, 1], wos[:, 1]))
            L.append((tri[:], tris[:]))
            L.append((xtiles[2][:], xs[:, 2]))
            L.append((xtiles[3][:], xs[:, 3]))
            rings = [nc.sync, nc.gpsimd, nc.scalar]
            for i, (dst, src) in enumerate(L):
                rings[i % 3].dma_start(dst, src)

            # ---- phase generators (each yield = one PE matmul) ----
            def gen1_qk(tci, cts):
                """Q/K projection chains for chunk tci, given ct list."""
                xt = xtiles[tci]
                for w_sb, dst in ((wq, qt), (wk, kt)):
                    for ct in cts:
                        ps = pmisc.tile([128, 512], F32, tag="ps")
                        for di in range(8):
                            nc.tensor.matmul(
                                ps[:],
                                w_sb[:, di, ct * 128 : (ct + 1) * 128],
                                xt[:, di, :],
                                start=(di == 0),
                                stop=(di == 7),
                            )
                            yield
                        nc.vector.tensor_copy(
                            dst[:, ct, tci * 512 : (tci + 1) * 512], ps[:]
                        )

            def gen1_v(tci):
                """V projection chains for chunk tci."""
                xt = xtiles[tci]
                for tt in range(4):
                    ps = pmisc.tile([128, GC], F32, tag="ps")
                    for di in range(8):
                        nc.tensor.matmul(
                            ps[:],
                            xt[:, di, tt * 128 : (tt + 1) * 128],
                            wv[:, di, :],
                            start=(di == 0),
                            stop=(di == 7),
                        )
                        yield
                    nc.vector.tensor_copy(
                        vp[:, tci * 4 + tt, :, 0:DH],
                        ps[:].rearrange("p (h d) -> p h d", h=HPC),
                    )

            def gen1(tci):
                """Q/K/V projections for chunk tci."""
                yield from gen1_qk(tci, (0, 1))
                yield from gen1_v(tci)

            def gen3(qc):
                """Out-projection partials for the 4 q-tiles of chunk qc."""
                for tt in range(4):
                    qti = qc * 4 + tt
                    ot = outp.tile([128, 2, 512], BF16, tag="ot")
                    for nn in range(2):
                        po = pmisc.tile(
                            [128, 512], F32, tag="ps", name=f"po{qti}_{nn}"
                        )
                        for ctt in range(2):
                            nc.tensor.matmul(
                                po,
                                at[:, ctt, qti * 128 : (qti + 1) * 128],
                                wo[:, ctt, nn * 512 : (nn + 1) * 512],
                                start=(ctt == 0),
                                stop=(ctt == 1),
                            )
                            yield
                        nc.vector.tensor_copy(ot[:, nn, :], po)
                    nc.sync.dma_start(
                        out[qti * 128 : (qti + 1) * 128, :].rearrange(
                            "q (a n) -> q a n", a=2
                        ),
                        ot[:],
                    )

            def epilogue3(mid_gen):
                """Chunk 3 out-projection: phase 2 is done, so spread the 8
                accumulators over all free PSUM banks; run the ctt=0 pass
                (at ct0 ready early) and the held-back chunk-2 out-proj while
                the tail normalization finishes, then ctt=1 + evictions."""
                b12 = pst.tile([128, 2, 512], F32, tag="st", name="ep12")
                b13 = pst.tile([128, 2, 512], F32, tag="st", name="ep13")
                po_of = {
                    (12, 0): b12[:, 0, :],
                    (12, 1): b12[:, 1, :],
                    (13, 0): b13[:, 0, :],
                    (13, 1): b13[:, 1, :],
                }
                for qti in range(12, 14):
                    for nn in range(2):
                        nc.tensor.matmul(
                            po_of[(qti, nn)],
                            at[:, 0, qti * 128 : (qti + 1) * 128],
                            wo[:, 0, nn * 512 : (nn + 1) * 512],
                            start=True,
                            stop=False,
                        )
                    tick()
                # held-back chunk-2 out-projection fills the norm latency
                # (pmisc ring is free to cycle here)
                for _ in mid_gen:
                    tick()
                a14 = [
                    pacc.tile([128, 512], F32, tag="oo", name="ep14_0"),
                    pacc.tile([128, 512], F32, tag="oo", name="ep14_1"),
                ]
                a15 = [
                    pmisc.tile([128, 512], F32, tag="ps", name="ep15_0"),
                    pmisc.tile([128, 512], F32, tag="ps", name="ep15_1"),
                ]
                po_of.update(
                    {
                        (14, 0): a14[0][:],
                        (14, 1): a14[1][:],
                        (15, 0): a15[0][:],
                        (15, 1): a15[1][:],
                    }
                )
                for qti in range(14, 16):
                    for nn in range(2):
                        nc.tensor.matmul(
                            po_of[(qti, nn)],
                            at[:, 0, qti * 128 : (qti + 1) * 128],
                            wo[:, 0, nn * 512 : (nn + 1) * 512],
                            start=True,
                            stop=False,
                        )
                    tick()
                dma_rings = [nc.sync, nc.gpsimd, nc.scalar]
                k = 0
                for qti in range(12, 16):
                    for nn in range(2):
                        nc.tensor.matmul(
                            po_of[(qti, nn)],
                            at[:, 1, qti * 128 : (qti + 1) * 128],
                            wo[:, 1, nn * 512 : (nn + 1) * 512],
                            start=False,
                            stop=True,
                        )
                    tick()
                    ot = outp.tile(
                        [128, 2, 512], BF16, tag="ot", name=f"otE{qti}"
                    )
                    for nn in range(2):
                        nc.vector.tensor_copy(ot[:, nn, :], po_of[(qti, nn)])
                        dma_rings[k % 3].dma_start(
                            out[
                                qti * 128 : (qti + 1) * 128,
                                nn * 512 : (nn + 1) * 512,
                            ],
                            ot[:, nn, :],
                        )
                        k += 1

            # deferred normalization stages: (groups_remaining, closure);
            # ticked once per phase-2 kti group, flushed before the epilogue
            pending = []

            def tick():
                for ent in pending[:]:
                    ent[0] -= 1
                    if ent[0] <= 0:
                        pending.remove(ent)
                        ent[1]()

            def flush_pending():
                while pending:
                    ent = pending.pop(0)
                    ent[1]()

            def phase2(qc, fillers, nfill):
                q0 = qc * 512
                n_kt = 4 * (qc + 1)
                total_slots = 2 * n_kt
                state = {"pumped": 0, "slot": 0}

                def pump():
                    state["slot"] += 1
                    want = (nfill * state["slot"]) // total_slots
                    while state["pumped"] < want:
                        try:
                            next(fillers)
                            state["pumped"] += 1
                        except StopIteration:
                            return

                for hp in range(2):
                    ct = hp
                    oo = [
                        pacc.tile([128, 512], F32, tag="oo", name=f"oo{hp}_0"),
                        pacc.tile([128, 512], F32, tag="oo", name=f"oo{hp}_1"),
                    ]

                    def emit_pv(pt_, kti_):
                        # forward kti order: kti 0 covers [0:512] (start); a
                        # diagonal kti is the LAST writer of its 128-block,
                        # so that block's piece carries the stop flag.
                        off = max(0, kti_ * 128 - q0)
                        first = kti_ == 0
                        for hh in range(2):
                            v_h = vp[:, kti_, 2 * hp + hh, :]
                            if kti_ >= 4 * qc:
                                nc.tensor.matmul(
                                    oo[hh][0 : DH + 1, off : off + 128],
                                    v_h,
                                    pt_[:, hh, off : off + 128],
                                    start=first,
                                    stop=True,
                                )
                                if off + 128 < 512:
                                    nc.tensor.matmul(
                                        oo[hh][0 : DH + 1, off + 128 : 512],
                                        v_h,
                                        pt_[:, hh, off + 128 : 512],
                                        start=first,
                                        stop=False,
                                    )
                            else:
                                nc.tensor.matmul(
                                    oo[hh][0 : DH + 1, :],
                                    v_h,
                                    pt_[:, hh, :],
                                    start=first,
                                    stop=False,
                                )

                    prev = None
                    for kti in range(n_kt):
                        off = max(0, kti * 128 - q0)
                        st = pst.tile([128, 2, 512], F32, tag="st")
                        ptile = ptp.tile([128, 2, 512], BF16, tag="pt")
                        for hh in range(2):
                            po = 64 * hh
                            nc.tensor.matmul(
                                st[:, hh, off:],
                                kt[po : po + 64, ct, kti * 128 : (kti + 1) * 128],
                                qt[po : po + 64, ct, q0 + off : q0 + 512],
                                start=True,
                                stop=True,
                            )
                        nc.scalar.activation(
                            ptile[:, :, off:], st[:, :, off:], EXP, scale=0.125
                        )
                        if kti >= 4 * qc:
                            # true diagonal 128-block: zero out q < k via mask
                            nc.vector.tensor_mul(
                                ptile[:, :, off : off + 128],
                                ptile[:, :, off : off + 128],
                                tri[:],
                            )
                        if prev is not None:
                            emit_pv(*prev)
                        prev = (ptile, kti)
                        tick()
                        pump()
                    emit_pv(*prev)
                    # normalization. Free both accumulators immediately (stg
                    # copies); defer the reciprocal and the at-multiply into
                    # the following kti stream, far enough that their
                    # cross-engine waits are already satisfied when the DVE
                    # queue reaches them.
                    stgs, s4s = [], []
                    for hh in range(2):
                        stg = normp.tile(
                            [DH + 1, 512], F32, tag="stg", name=f"stg{hh}"
                        )
                        nc.vector.tensor_copy(stg[:], oo[hh][0 : DH + 1, :])
                        s4 = normp.tile([4, 128], F32, tag="s4", name=f"s4{hh}")
                        nc.sync.dma_start(s4[:], stg[DH : DH + 1, :])
                        stgs.append(stg)
                        s4s.append(s4)

                    # Four no-wait stages: each fires only after its input is
                    # already complete, so queue-blocking waits never happen.
                    # Delays are in kti-groups (group ~2.6us for qc0, ~1.1us
                    # after; epilogue ticks ~0.4us).
                    if qc == 0 and hp == 0:
                        d_rc, d_rr, d_bc, d_mu = 2, 3, 4, 5
                    elif qc == 3 and hp == 1:
                        d_rc, d_rr, d_bc, d_mu = 7, 9, 13, 18
                    else:
                        d_rc, d_rr, d_bc, d_mu = 3, 4, 6, 8

                    rrs, rbs = [], []
                    for hh in range(2):
                        rrs.append(
                            normp.tile([1, 512], F32, tag="rr", name=f"rr{hh}")
                        )
                        rbs.append(
                            normp.tile(
                                [64, 512], F32, tag="rb", name=f"rb{hh}"
                            )
                        )

                    def st_recip(s4s=s4s):
                        for hh in range(2):
                            nc.vector.reciprocal(s4s[hh][:], s4s[hh][:])

                    def st_rrdma(s4s=s4s, rrs=rrs):
                        for hh in range(2):
                            nc.sync.dma_start(rrs[hh][:], s4s[hh][:])

                    def st_bcast(rrs=rrs, rbs=rbs):
                        for hh in range(2):
                            nc.gpsimd.partition_broadcast(rbs[hh][:], rrs[hh][:])

                    def st_mul(lo, hi, ct=ct, q0=q0, stgs=stgs, rbs=rbs):
                        for hh in range(2):
                            po = 64 * hh
                            nc.vector.tensor_mul(
                                at[po : po + 64, ct, q0 + lo : q0 + hi],
                                stgs[hh][0:DH, lo:hi],
                                rbs[hh][:, lo:hi],
                            )

                    pending.append([d_rc, st_recip])
                    pending.append([d_rr, st_rrdma])
                    pending.append([d_bc, st_bcast])
                    if qc == 3 and hp == 1:
                        pending.append([15, lambda f=st_mul: f(0, 256)])
                        pending.append([18, lambda f=st_mul: f(256, 512)])
                    else:
                        pending.append([d_mu, lambda f=st_mul: f(0, 512)])
                # drain leftover fillers
                for _ in fillers:
                    pass

            def chain(*gens):
                for g in gens:
                    yield from g

            for _ in gen1(0):
                pass
            phase2(0, chain(gen1(1)), 64)
            phase2(1, chain(gen1(2)), 64)
            phase2(2, chain(gen1(3)), 64)
            phase2(3, chain(gen3(0), gen3(1)), 32)
            epilogue3(gen3(2))
            flush_pending()
    nc.compile()
    return nc


def _get_nc():
    if "nc" not in _CACHE:
        _CACHE["nc"] = _build()
    return _CACHE["nc"]


def _in_maps(x, Wq, Wk, Wv, Wo):
    x = np.asarray(x, dtype=np.float32)
    Wq = np.asarray(Wq, dtype=np.float32)
    Wk = np.asarray(Wk, dtype=np.float32)
    Wv = np.asarray(Wv, dtype=np.float32)
    Wo = np.asarray(Wo, dtype=np.float32)
    maps = []
    for core in range(N_CORES):
        b, g = divmod(core, 4)
        sl = slice(g * GC, (g + 1) * GC)
        # xs[p, tc, dt, t] = x[b, tc*512+t, dt*128+p]
        xsw = np.ascontiguousarray(
            x[b].reshape(4, 512, 8, 128).transpose(3, 0, 2, 1)
        ).astype(NPBF)
        # w[p, dt, c] = W[sl][c, dt*128+p]
        wqw = np.ascontiguousarray(
            Wq[sl].reshape(GC, 8, 128).transpose(2, 1, 0)
        ).astype(NPBF)
        wkw = np.ascontiguousarray(
            Wk[sl].reshape(GC, 8, 128).transpose(2, 1, 0)
        ).astype(NPBF)
        wvw = np.ascontiguousarray(
            Wv[sl].reshape(GC, 8, 128).transpose(2, 1, 0)
        ).astype(NPBF)
        # wo[p, ct, n] = Wo[n, g*256 + ct*128 + p]
        wow = np.ascontiguousarray(
            Wo[:, sl].reshape(D, 2, 128).transpose(2, 1, 0)
        ).astype(NPBF)
        # tris[p, a, j] = 1 if j >= p (q >= k within the diagonal block)
        j = np.arange(128)
        trim = (j[None, :] >= j[:, None]).astype(NPBF)
        trisw = np.ascontiguousarray(np.repeat(trim[:, None, :], 2, axis=1))
        maps.append(
            {
                "xs": xsw,
                "wqs": wqw,
                "wks": wkw,
                "wvs": wvw,
                "wos": wow,
                "tris": trisw,
            }
        )
    return maps


def _run(x, Wq, Wk, Wv, Wo, **spmd_kwargs):
    nc = _get_nc()
    res = run_bass_kernel_spmd(
        nc, _in_maps(x, Wq, Wk, Wv, Wo), core_ids=list(range(N_CORES)), **spmd_kwargs
    )
    outs = [np.asarray(r["out"], dtype=np.float32) for r in res.results]
    full = np.stack(
        [
            outs[0] + outs[1] + outs[2] + outs[3],
            outs[4] + outs[5] + outs[6] + outs[7],
        ]
    ).astype(np.float32)
    return full, res


def kernel(x, Wq, Wk, Wv, Wo):
    full, _ = _run(x, Wq, Wk, Wv, Wo)
    return full
